# revision 8
# baseline (speedup 1.0000x reference)
"""Trainium2 Bass kernel for a causal multi-head attention layer.

Model: b=2, s=2048, d_model=1024, 16 heads, head_dim=64, pad-index 0.
Sharding over 8 NeuronCores: each core owns 2 heads (128 of the 1024
attention dims) for both batches (head/tensor parallel).  After attention,
AllToAlls redistribute the per-head outputs so each core holds all 1024
attention dims for 1/8 of the sequence positions, where it runs the output
projection locally.  Output rows per core: 512 (4 chunks of 128).

Schedule: the exp of the attention scores (ACT engine, ~88us for both
batches) is the critical chain, and the PE stream blocks on the 2-deep
score-PSUM ring whenever it runs more than 2 score entries ahead of the
ACT engine.  So the emitter rate-matches: score entries are woven one at
a time between ~1us micro-units of projection / PV / output-projection
work.  A2As run as 4 collectives (one per half-batch) fired as soon as
each half is normalized; a dummy collective at the top absorbs the
collective-stream warmup and core-launch skew.
"""

import threading

import numpy as np

B, S, D = 2, 2048, 1024
H, HD = 16, 64
NCORES = 8
LD = D // NCORES          # 128 local attention dims (2 heads)
R = B * S                 # 4096 flattened rows
RC = R // NCORES          # 512 output rows per core
NKT = S // 128            # 16 key tiles per batch
NCH = D // 128            # 8 contraction chunks of d_model
NST = S // 512            # 4 query stripes per batch

_cache = {}
_lock = threading.Lock()


def _stripe_layout():
    """Per stripe c: list of (kt, width, q_start, offset-in-block), block len."""
    layout = []
    for c in range(NST):
        entries = []
        off = 0
        for kt in range(4 * c + 4):
            qs = max(512 * c, kt * 128)
            w = 512 * (c + 1) - qs
            entries.append((kt, w, qs, off))
            off += w
        layout.append((entries, off))
    return layout


def _build_nc():
    import concourse.mybir as mybir
    import concourse.tile as tile
    from concourse import bacc
    from contextlib import ExitStack

    f32 = mybir.dt.float32
    bf16 = mybir.dt.bfloat16
    i32 = mybir.dt.int32
    AF = mybir.ActivationFunctionType
    ALU = mybir.AluOpType

    nc = bacc.Bacc(None, target_bir_lowering=False, num_devices=NCORES)

    xT = nc.declare_dram_parameter("xT", [D, R], bf16, isOutput=False)
    wqT = nc.declare_dram_parameter("wqT", [D, LD], bf16, isOutput=False)
    wkT = nc.declare_dram_parameter("wkT", [D, LD], bf16, isOutput=False)
    wvT = nc.declare_dram_parameter("wvT", [D, LD], bf16, isOutput=False)
    woT = nc.declare_dram_parameter("woT", [D, D], bf16, isOutput=False)
    bq = nc.declare_dram_parameter("bq", [LD], f32, isOutput=False)
    bk = nc.declare_dram_parameter("bk", [LD], f32, isOutput=False)
    bv = nc.declare_dram_parameter("bv", [LD], f32, isOutput=False)
    bo = nc.declare_dram_parameter("bo", [D], f32, isOutput=False)
    ids = nc.declare_dram_parameter("ids", [128, B * NKT], i32, isOutput=False)
    out = nc.declare_dram_parameter("out", [RC, D], f32, isOutput=True)

    layout = _stripe_layout()

    with ExitStack() as ctx:
        tc = ctx.enter_context(tile.TileContext(nc))
        const = ctx.enter_context(tc.tile_pool(name="const", bufs=1))
        xcp = ctx.enter_context(tc.tile_pool(name="xcp", bufs=1))
        qkp = ctx.enter_context(tc.tile_pool(name="qkp", bufs=2))
        estp = ctx.enter_context(tc.tile_pool(name="estp", bufs=1))
        stg = ctx.enter_context(tc.tile_pool(name="stg", bufs=2))
        work = ctx.enter_context(tc.tile_pool(name="work", bufs=2))
        recp = ctx.enter_context(tc.tile_pool(name="recp", bufs=1))
        ppool = ctx.enter_context(tc.tile_pool(name="ppool", bufs=2, space="PSUM"))
        spool = ctx.enter_context(tc.tile_pool(name="spool", bufs=2, space="PSUM"))
        pvpool = ctx.enter_context(tc.tile_pool(name="pvpool", bufs=2, space="PSUM"))
        dpool = ctx.enter_context(tc.tile_pool(name="dram", bufs=4, space="DRAM"))

        # ---- dummy collective first: syncs the cores and absorbs the
        # collective-stream warmup while the compute phase runs ----
        dummy_i = dpool.tile([8, 16], bf16, name="dummy_i", tag="dummy_i")
        dummy_o = dpool.tile([8, 16], bf16, name="dummy_o", tag="dummy_o")
        nc.gpsimd.collective_compute(
            "AllToAll", ALU.bypass, replica_groups=[list(range(NCORES))],
            ins=[dummy_i.opt()], outs=[dummy_o.opt()])

        # ---- constants on the GpSimd DMA queue so the x loads own Sync ----
        wqT_sb = const.tile([128, NCH, LD], bf16)
        nc.gpsimd.dma_start(wqT_sb, wqT.ap().rearrange("(c p) d -> p c d", p=128))
        wkT_sb = const.tile([128, NCH, LD], bf16)
        nc.gpsimd.dma_start(wkT_sb, wkT.ap().rearrange("(c p) d -> p c d", p=128))
        bq_col = const.tile([128, 1], f32)
        nc.gpsimd.dma_start(bq_col, bq.ap().rearrange("(p o) -> p o", o=1))
        bk_col = const.tile([128, 1], f32)
        nc.gpsimd.dma_start(bk_col, bk.ap().rearrange("(p o) -> p o", o=1))
        wvT_sb = const.tile([128, NCH, LD], bf16)
        nc.gpsimd.dma_start(wvT_sb, wvT.ap().rearrange("(c p) d -> p c d", p=128))
        bv_bc = const.tile([128, LD], f32)
        nc.gpsimd.dma_start(bv_bc, bv.ap().partition_broadcast(128))
        ids_sb = const.tile([128, B * NKT], i32)
        nc.gpsimd.dma_start(ids_sb, ids.ap())
        woT_sb = const.tile([128, NCH, D], bf16)
        nc.gpsimd.dma_start(woT_sb, woT.ap().rearrange("(c p) n -> p c n", p=128))
        bo_bc = const.tile([128, D], f32)
        nc.gpsimd.dma_start(bo_bc, bo.ap().partition_broadcast(128))

        ones64 = const.tile([1, 64], bf16)
        nc.vector.memset(ones64, 1.0)

        # x^T in one [128, c, r] tile; one big DMA per 512-row block so
        # the Sync queue issues 4 descriptors per batch instead of 32
        xTr = xT.ap().rearrange("(c p) r -> p c r", p=128)
        xc = xcp.tile([128, NCH, S], bf16, name="xc", tag="xc")

        def xc_load(b):
            for rb in range(4):
                rsl = slice(rb * 512, (rb + 1) * 512)
                dsl = slice(b * S + rb * 512, b * S + (rb + 1) * 512)
                nc.sync.dma_start(xc[:, :, rsl], xTr[:, :, dsl])

        xc_load(0)

        padf = const.tile([128, B * NKT], f32)
        nc.vector.tensor_copy(padf, ids_sb)
        nc.vector.tensor_scalar_min(padf, padf, 1.0)

        # diagmask2[x, h, y] = 1 if y >= x else 0 (keys on partitions)
        diagmask = const.tile([128, 128], bf16)
        nc.gpsimd.memset(diagmask, 1.0)
        nc.gpsimd.affine_select(
            out=diagmask, in_=diagmask, compare_op=ALU.is_ge, fill=0.0,
            base=0, pattern=[[1, 128]], channel_multiplier=-1,
        )
        diagmask2 = const.tile([128, 2, 128], bf16)
        nc.vector.tensor_copy(diagmask2[:, 0, :], diagmask)
        nc.vector.tensor_copy(diagmask2[:, 1, :], diagmask)

        # ---- per-batch persistent tiles ----
        qt = {}
        kt_ = {}
        vaug = {}
        stage = {}
        ests = {}
        pos = {}
        recbs = {}
        a2a_outs = {}

        EST_BUFS = [2, 2, 1, 1]

        def get_batch_tiles(b):
            if b in qt:
                return
            qt[b] = qkp.tile([128, S], bf16, name=f"qt{b}", tag="qt")
            kt_[b] = qkp.tile([128, S], bf16, name=f"kt{b}", tag="kt")
            vaug[b] = qkp.tile([128, 2, NKT, HD + 1], bf16, name=f"vaug{b}",
                               tag="vaug")
            stage[b] = stg.tile([128, S], bf16, name=f"stage{b}", tag="stage")
            ests[b] = [estp.tile([128, 2, blocklen], bf16, name=f"est{c}",
                                 tag=f"est{c}", bufs=EST_BUFS[c])
                       for c, (_, blocklen) in enumerate(layout)]

        # ---- score entries (the ACT-paced stream) ----
        sc_ready = []
        act_cost = [0.0]
        pe_cost = [0.0]

        def sc_emit():
            b, c, e = sc_ready.pop(0)
            kt, w, qs, off = layout[c][0][e]
            est = ests[b][c]
            ksl = slice(kt * 128, (kt + 1) * 128)
            ps = spool.tile([128, 2, 512], f32, name="ps", tag="sp")
            nc.tensor.matmul(ps[:, 0, 0:w], kt_[b][0:64, ksl],
                             qt[b][0:64, qs:qs + w], start=True, stop=True)
            nc.tensor.matmul(ps[:, 1, 0:w], kt_[b][64:128, ksl],
                             qt[b][64:128, qs:qs + w], start=True, stop=True)
            nc.scalar.activation(est[:, :, off:off + w], ps[:, :, 0:w],
                                 AF.Exp, scale=0.125)
            if kt >= 4 * c:  # diagonal tile: causal mask
                nc.vector.tensor_mul(est[:, :, off:off + 128],
                                     est[:, :, off:off + 128], diagmask2)
            act_cost[0] += 2 * w * 0.00109 + 0.1

        def enq(b, c):
            for e in range(len(layout[c][0])):
                sc_ready.append((b, c, e))

        def pump():
            while sc_ready and act_cost[0] < pe_cost[0] + 2.2:
                sc_emit()

        def force(b, c):
            while sc_ready and sc_ready[0][:2] <= (b, c):
                sc_emit()

        def fill(us, fn, *args):
            fn(*args)
            pe_cost[0] += us
            pump()

        # ---- micro-unit worklets ----
        qk_state = {}

        def qk_unit(b, rb, j):
            """Quarter of a q/k projection row-block: chunks 2j, 2j+1."""
            get_batch_tiles(b)
            rsl = slice(rb * 512, (rb + 1) * 512)
            if j == 0:
                qk_state['pqt'] = ppool.tile([128, 512], f32, name="pqt",
                                             tag="pp")
                qk_state['pkt'] = ppool.tile([128, 512], f32, name="pkt",
                                             tag="pp")
            pqt, pkt = qk_state['pqt'], qk_state['pkt']
            for c in (2 * j, 2 * j + 1):
                st = c == 0
                sp = c == NCH - 1
                rhs = xc[:, c, rsl]
                nc.tensor.matmul(pqt, wqT_sb[:, c, :], rhs, start=st, stop=sp)
                nc.tensor.matmul(pkt, wkT_sb[:, c, :], rhs, start=st, stop=sp)
            if j == 3:
                nc.vector.tensor_scalar_add(qt[b][:, rsl], pqt, bq_col)
                nc.vector.tensor_scalar_add(kt_[b][:, rsl], pkt, bk_col)

        def v_unit(b, m0):
            """Two V m-tiles (keys 128*m0 .. 128*m0+256)."""
            for m in (m0, m0 + 1):
                msl = slice(m * 128, (m + 1) * 128)
                pv_ = ppool.tile([128, LD], f32, name="pv", tag="pp")
                for c in range(NCH):
                    nc.tensor.matmul(pv_, xc[:, c, msl], wvT_sb[:, c, :],
                                     start=(c == 0), stop=(c == NCH - 1))
                tv = work.tile([128, LD], f32, name="tv", tag="tv")
                nc.vector.tensor_add(tv, pv_, bv_bc)
                pcol = padf[:, b * NKT + m:b * NKT + m + 1]
                for h in range(2):
                    nc.vector.tensor_scalar_mul(
                        vaug[b][:, h, m, 0:HD], tv[:, h * HD:(h + 1) * HD],
                        pcol)
                    nc.vector.tensor_copy(vaug[b][:, h, m, HD:HD + 1], pcol)

        def pv(b, c):
            entries, _ = layout[c]
            est = ests[b][c]
            for h in range(2):
                po = pvpool.tile([128, 512], f32, name=f"po{h}", tag="po")
                pos[(b, c, h)] = po
                last = 4 * c + 3
                for kt, w, qs, off in entries:
                    po_off = qs - 512 * c
                    nc.tensor.matmul(po[0:HD + 1, po_off:po_off + w],
                                     vaug[b][:, h, kt, :],
                                     est[:, h, off:off + w],
                                     start=(kt == 0), stop=(kt == last))
                den = recp.tile([1, 512], f32, name="den", tag=f"den{h}")
                nc.vector.tensor_copy(den, po[HD:HD + 1, :])
                rec = recp.tile([1, 512], f32, name="rec", tag=f"rec{h}")
                nc.vector.reciprocal_approx_fast(rec, den)
                recb = recp.tile([1, 512], bf16, name="recb",
                                 tag=f"recb{h}", bufs=2)
                nc.vector.tensor_copy(recb, rec)
                recbs[(b, c, h)] = recb

        def div(b, c):
            for h in range(2):
                nc.tensor.matmul(pos[(b, c, h)][64:128, :], ones64,
                                 recbs[(b, c, h)], start=True, stop=True,
                                 skip_group_check=True)
            for h in range(2):
                po = pos[(b, c, h)]
                rbc = recp.tile([HD, 512], bf16, name="rbc", tag=f"rbc{h}")
                nc.vector.tensor_copy(rbc, po[64:128, :])
                nc.vector.tensor_mul(
                    stage[b][h * HD:(h + 1) * HD, 512 * c:512 * (c + 1)],
                    po[0:HD, :], rbc)

        def a2a(b, h2):
            q0, q1 = 1024 * h2, 1024 * (h2 + 1)
            nq = (q1 - q0) // NCORES
            a2a_in = dpool.tile([NCORES * 128, nq], bf16,
                                name=f"a2ai{b}{h2}", tag="a2ai")
            nc.gpsimd.dma_start(
                a2a_in.rearrange("(j p) r -> p j r", p=128),
                stage[b][:, q0:q1].rearrange("p (j r) -> p j r", j=NCORES))
            a2a_out = dpool.tile([NCORES * 128, nq], bf16,
                                 name=f"a2ao{b}{h2}", tag="a2ao")
            nc.gpsimd.collective_compute(
                "AllToAll", ALU.bypass,
                replica_groups=[list(range(NCORES))],
                ins=[a2a_in.opt()], outs=[a2a_out.opt()])
            a2a_outs[(b, h2)] = a2a_out

        op_state = {}

        def op_unit(b, h2, n):
            """Half an output-projection chunk (512 of 1024 out dims)."""
            if n == 0:
                a2a_sb = stg.tile([128, NCORES, 128], bf16,
                                  name=f"a2as{b}{h2}", tag="a2as", bufs=3)
                nc.sync.dma_start(
                    a2a_sb,
                    a2a_outs[(b, h2)].rearrange("(j p) r -> p j r", p=128))
                op_state[(b, h2)] = a2a_sb
            a2a_sb = op_state[(b, h2)]
            r0 = (2 * b + h2) * 128
            pout = ppool.tile([128, 512], f32, name="pout", tag="pp")
            for c in range(NCH):
                nc.tensor.matmul(
                    pout, a2a_sb[:, c, :],
                    woT_sb[:, c, n * 512:(n + 1) * 512],
                    start=(c == 0), stop=(c == NCH - 1))
            ot = work.tile([128, 512], f32, name="ot", tag="ot")
            nc.vector.tensor_add(ot, pout, bo_bc[:, n * 512:(n + 1) * 512])
            nc.sync.dma_start(
                out.ap()[r0:r0 + 128, n * 512:(n + 1) * 512], ot)

        # ---- emission schedule ----
        QKU, VU, OPU, DIVU = 1.05, 1.1, 2.1, 0.55

        for rb in range(4):
            for j in range(4):
                fill(QKU, qk_unit, 0, rb, j)
            enq(0, rb)
            pump()
        for m0 in range(0, NKT, 2):
            fill(VU, v_unit, 0, m0)
        xc_load(1)

        for j in range(4):
            fill(QKU, qk_unit, 1, 0, j)
        force(0, 0)
        fill(3.0, pv, 0, 0)
        for j in range(4):
            fill(QKU, qk_unit, 1, 1, j)
        fill(DIVU, div, 0, 0)
        force(0, 1)
        fill(3.0, pv, 0, 1)
        for j in range(4):
            fill(QKU, qk_unit, 1, 2, j)
        fill(DIVU, div, 0, 1)
        a2a(0, 0)
        enq(1, 0)
        enq(1, 1)
        force(0, 2)
        fill(4.5, pv, 0, 2)
        for j in range(4):
            fill(QKU, qk_unit, 1, 3, j)
        fill(DIVU, div, 0, 2)
        force(0, 3)
        fill(6.5, pv, 0, 3)
        for m0 in range(0, 8, 2):
            fill(VU, v_unit, 1, m0)
        fill(DIVU, div, 0, 3)
        a2a(0, 1)
        enq(1, 2)
        for m0 in range(8, NKT, 2):
            fill(VU, v_unit, 1, m0)
        force(1, 0)
        fill(1.2, pv, 1, 0)
        fill(DIVU, div, 1, 0)
        force(1, 1)
        fill(3.0, pv, 1, 1)
        fill(DIVU, div, 1, 1)
        a2a(1, 0)
        enq(1, 3)
        pump()
        force(1, 2)
        fill(4.5, pv, 1, 2)
        fill(DIVU, div, 1, 2)
        fill(OPU, op_unit, 0, 0, 0)
        fill(OPU, op_unit, 0, 0, 1)
        fill(OPU, op_unit, 0, 1, 0)
        fill(OPU, op_unit, 0, 1, 1)
        force(1, 3)
        fill(6.5, pv, 1, 3)
        fill(DIVU, div, 1, 3)
        a2a(1, 1)
        fill(OPU, op_unit, 1, 0, 0)
        fill(OPU, op_unit, 1, 0, 1)
        fill(OPU, op_unit, 1, 1, 0)
        fill(OPU, op_unit, 1, 1, 1)

        assert not sc_ready

    nc.finalize()
    return nc


def _get_nc():
    with _lock:
        if "nc" not in _cache:
            _cache["nc"] = _build_nc()
        return _cache["nc"]


def _shard_inputs(x, input_ids, Wq, bq, Wk, bk, Wv, bv, Wo, bo):
    import ml_dtypes
    bf16 = ml_dtypes.bfloat16

    x = np.asarray(x, dtype=np.float32)
    xT = np.ascontiguousarray(x.reshape(R, D).T).astype(bf16)
    woT = np.ascontiguousarray(np.asarray(Wo, dtype=np.float32).T).astype(bf16)
    bo_f = np.asarray(bo, dtype=np.float32)
    ids = np.asarray(input_ids).astype(np.int32)
    # ids_r[p, b*NKT + t] = input_ids[b, t*128 + p]
    ids_r = np.ascontiguousarray(ids.reshape(B, NKT, 128).transpose(2, 0, 1)
                                 .reshape(128, B * NKT))
    Wq = np.asarray(Wq, dtype=np.float32)
    Wk = np.asarray(Wk, dtype=np.float32)
    Wv = np.asarray(Wv, dtype=np.float32)
    bq = np.asarray(bq, dtype=np.float32)
    bk = np.asarray(bk, dtype=np.float32)
    bv = np.asarray(bv, dtype=np.float32)

    in_maps = []
    for c in range(NCORES):
        sl = slice(c * LD, (c + 1) * LD)
        in_maps.append({
            "xT": xT,
            "wqT": np.ascontiguousarray(Wq[sl].T).astype(bf16),
            "wkT": np.ascontiguousarray(Wk[sl].T).astype(bf16),
            "wvT": np.ascontiguousarray(Wv[sl].T).astype(bf16),
            "woT": woT,
            "bq": bq[sl].copy(),
            "bk": bk[sl].copy(),
            "bv": bv[sl].copy(),
            "bo": bo_f,
            "ids": ids_r,
        })
    return in_maps


def run(trace=False, **inputs):
    """Run the kernel; returns (output, BassKernelResults)."""
    from concourse.bass_utils import run_bass_kernel_spmd

    nc = _get_nc()
    in_maps = _shard_inputs(**inputs)
    res = run_bass_kernel_spmd(nc, in_maps, core_ids=list(range(NCORES)),
                               trace=trace)
    full = np.empty((B, S, D), dtype=np.float32)
    for j in range(NCORES):
        o = np.asarray(res.results[j]["out"], dtype=np.float32)
        for b in range(B):
            for h2 in range(2):
                full[b, 1024 * h2 + 128 * j:1024 * h2 + 128 * (j + 1), :] = \
                    o[(2 * b + h2) * 128:(2 * b + h2 + 1) * 128, :]
    return full, res


def kernel(**inputs) -> np.ndarray:
    full, _ = run(trace=False, **inputs)
    return full


# revision 11
# speedup vs baseline: 1.0617x; 1.0617x over previous
"""Trainium2 Bass kernel for a causal multi-head attention layer.

Model: b=2, s=2048, d_model=1024, 16 heads, head_dim=64, pad-index 0.
Sharding over 8 NeuronCores: each core owns 2 heads (128 of the 1024
attention dims) for both batches (head/tensor parallel).  After attention,
AllToAlls redistribute the per-head outputs so each core holds all 1024
attention dims for 1/8 of the sequence positions, where it runs the output
projection locally.  Output rows per core: 512 (4 chunks of 128).

Schedule: the exp of the attention scores (ACT engine, ~88us for both
batches) is the critical chain, and the PE stream blocks on the 2-deep
score-PSUM ring whenever it runs more than 2 score entries ahead of the
ACT engine.  So the emitter rate-matches: score entries are woven one at
a time between ~1us micro-units of projection / PV / output-projection
work.  A2As run as 4 collectives (one per half-batch) fired as soon as
each half is normalized; a dummy collective at the top absorbs the
collective-stream warmup and core-launch skew.
"""

import threading

import numpy as np

B, S, D = 2, 2048, 1024
H, HD = 16, 64
NCORES = 8
LD = D // NCORES          # 128 local attention dims (2 heads)
R = B * S                 # 4096 flattened rows
RC = R // NCORES          # 512 output rows per core
NKT = S // 128            # 16 key tiles per batch
NCH = D // 128            # 8 contraction chunks of d_model
NST = S // 512            # 4 query stripes per batch

_cache = {}
_lock = threading.Lock()


def _stripe_layout():
    """Per stripe c: list of (kt, width, q_start, offset-in-block), block len."""
    layout = []
    for c in range(NST):
        entries = []
        off = 0
        for kt in range(4 * c + 4):
            qs = max(512 * c, kt * 128)
            w = 512 * (c + 1) - qs
            entries.append((kt, w, qs, off))
            off += w
        layout.append((entries, off))
    return layout


def _build_nc():
    import concourse.mybir as mybir
    import concourse.tile as tile
    from concourse import bacc
    from contextlib import ExitStack

    f32 = mybir.dt.float32
    bf16 = mybir.dt.bfloat16
    i32 = mybir.dt.int32
    AF = mybir.ActivationFunctionType
    ALU = mybir.AluOpType

    nc = bacc.Bacc(None, target_bir_lowering=False, num_devices=NCORES)

    xT = nc.declare_dram_parameter("xT", [D, R], bf16, isOutput=False)
    wqT = nc.declare_dram_parameter("wqT", [D, LD], bf16, isOutput=False)
    wkT = nc.declare_dram_parameter("wkT", [D, LD], bf16, isOutput=False)
    wvT = nc.declare_dram_parameter("wvT", [D, LD], bf16, isOutput=False)
    woT = nc.declare_dram_parameter("woT", [D, D], bf16, isOutput=False)
    bq = nc.declare_dram_parameter("bq", [LD], f32, isOutput=False)
    bk = nc.declare_dram_parameter("bk", [LD], f32, isOutput=False)
    bv = nc.declare_dram_parameter("bv", [LD], f32, isOutput=False)
    bo = nc.declare_dram_parameter("bo", [D], f32, isOutput=False)
    ids = nc.declare_dram_parameter("ids", [128, B * NKT], i32, isOutput=False)
    out = nc.declare_dram_parameter("out", [RC, D], f32, isOutput=True)

    layout = _stripe_layout()

    with ExitStack() as ctx:
        tc = ctx.enter_context(tile.TileContext(nc))
        const = ctx.enter_context(tc.tile_pool(name="const", bufs=1))
        xcp = ctx.enter_context(tc.tile_pool(name="xcp", bufs=1))
        qkp = ctx.enter_context(tc.tile_pool(name="qkp", bufs=2))
        estp = ctx.enter_context(tc.tile_pool(name="estp", bufs=1))
        stg = ctx.enter_context(tc.tile_pool(name="stg", bufs=2))
        work = ctx.enter_context(tc.tile_pool(name="work", bufs=2))
        recp = ctx.enter_context(tc.tile_pool(name="recp", bufs=1))
        ppool = ctx.enter_context(tc.tile_pool(name="ppool", bufs=2, space="PSUM"))
        spool = ctx.enter_context(tc.tile_pool(name="spool", bufs=2, space="PSUM"))
        pvpool = ctx.enter_context(tc.tile_pool(name="pvpool", bufs=2, space="PSUM"))
        dpool = ctx.enter_context(tc.tile_pool(name="dram", bufs=4, space="DRAM"))

        # ---- dummy collective first: syncs the cores and absorbs the
        # collective-stream warmup while the compute phase runs ----
        dummy_i = dpool.tile([8, 16], bf16, name="dummy_i", tag="dummy_i")
        dummy_o = dpool.tile([8, 16], bf16, name="dummy_o", tag="dummy_o")
        nc.gpsimd.collective_compute(
            "AllToAll", ALU.bypass, replica_groups=[list(range(NCORES))],
            ins=[dummy_i.opt()], outs=[dummy_o.opt()])

        # ---- constants on the GpSimd DMA queue so the x loads own Sync ----
        wqT_sb = const.tile([128, NCH, LD], bf16)
        nc.gpsimd.dma_start(wqT_sb, wqT.ap().rearrange("(c p) d -> p c d", p=128))
        wkT_sb = const.tile([128, NCH, LD], bf16)
        nc.gpsimd.dma_start(wkT_sb, wkT.ap().rearrange("(c p) d -> p c d", p=128))
        bq_col = const.tile([128, 1], f32)
        nc.gpsimd.dma_start(bq_col, bq.ap().rearrange("(p o) -> p o", o=1))
        bk_col = const.tile([128, 1], f32)
        nc.gpsimd.dma_start(bk_col, bk.ap().rearrange("(p o) -> p o", o=1))
        wvT_sb = const.tile([128, NCH, LD], bf16)
        nc.gpsimd.dma_start(wvT_sb, wvT.ap().rearrange("(c p) d -> p c d", p=128))
        bv_bc = const.tile([128, LD], f32)
        nc.gpsimd.dma_start(bv_bc, bv.ap().partition_broadcast(128))
        ids_sb = const.tile([128, B * NKT], i32)
        nc.gpsimd.dma_start(ids_sb, ids.ap())
        woT_sb = const.tile([128, NCH, D], bf16)
        nc.gpsimd.dma_start(woT_sb, woT.ap().rearrange("(c p) n -> p c n", p=128))
        bo_bc = const.tile([128, D], f32)
        nc.gpsimd.dma_start(bo_bc, bo.ap().partition_broadcast(128))

        ones64 = const.tile([1, 64], bf16)
        nc.vector.memset(ones64, 1.0)

        # x^T in one [128, c, r] tile; one big DMA per 512-row block so
        # the Sync queue issues 4 descriptors per batch instead of 32
        xTr = xT.ap().rearrange("(c p) r -> p c r", p=128)
        xc = xcp.tile([128, NCH, S], bf16, name="xc", tag="xc")

        def xc_load_rb(b, rb):
            rsl = slice(rb * 512, (rb + 1) * 512)
            dsl = slice(b * S + rb * 512, b * S + (rb + 1) * 512)
            nc.sync.dma_start(xc[:, :, rsl], xTr[:, :, dsl])

        for rb in range(4):
            xc_load_rb(0, rb)

        padf = const.tile([128, B * NKT], f32)
        nc.vector.tensor_copy(padf, ids_sb)
        nc.vector.tensor_scalar_min(padf, padf, 1.0)

        # diagmask2[x, h, y] = 1 if y >= x else 0 (keys on partitions)
        diagmask = const.tile([128, 128], bf16)
        nc.gpsimd.memset(diagmask, 1.0)
        nc.gpsimd.affine_select(
            out=diagmask, in_=diagmask, compare_op=ALU.is_ge, fill=0.0,
            base=0, pattern=[[1, 128]], channel_multiplier=-1,
        )
        diagmask2 = const.tile([128, 2, 128], bf16)
        nc.vector.tensor_copy(diagmask2[:, 0, :], diagmask)
        nc.vector.tensor_copy(diagmask2[:, 1, :], diagmask)

        # ---- per-batch persistent tiles ----
        qt = {}
        kt_ = {}
        vaug = {}
        stage = {}
        ests = {}
        pos = {}
        recbs = {}
        a2a_outs = {}

        EST_BUFS = [2, 2, 1, 1]

        def get_batch_tiles(b):
            if b in qt:
                return
            qt[b] = qkp.tile([128, S], bf16, name=f"qt{b}", tag="qt")
            kt_[b] = qkp.tile([128, S], bf16, name=f"kt{b}", tag="kt")
            vaug[b] = qkp.tile([128, 2, NKT, HD + 1], bf16, name=f"vaug{b}",
                               tag="vaug")
            stage[b] = stg.tile([128, S], bf16, name=f"stage{b}", tag="stage")
            ests[b] = [estp.tile([128, 2, blocklen], bf16, name=f"est{c}",
                                 tag=f"est{c}", bufs=EST_BUFS[c])
                       for c, (_, blocklen) in enumerate(layout)]

        # ---- score entries (the ACT-paced stream) ----
        sc_ready = []
        act_cost = [0.0]
        pe_cost = [0.0]

        def sc_emit():
            b, c, e = sc_ready.pop(0)
            kt, w, qs, off = layout[c][0][e]
            est = ests[b][c]
            ksl = slice(kt * 128, (kt + 1) * 128)
            ps = spool.tile([128, 2, 512], f32, name="ps", tag="sp")
            nc.tensor.matmul(ps[:, 0, 0:w], kt_[b][0:64, ksl],
                             qt[b][0:64, qs:qs + w], start=True, stop=True)
            nc.tensor.matmul(ps[:, 1, 0:w], kt_[b][64:128, ksl],
                             qt[b][64:128, qs:qs + w], start=True, stop=True)
            nc.scalar.activation(est[:, :, off:off + w], ps[:, :, 0:w],
                                 AF.Exp, scale=0.125)
            if kt >= 4 * c:  # diagonal tile: causal mask
                nc.vector.tensor_mul(est[:, :, off:off + 128],
                                     est[:, :, off:off + 128], diagmask2)
            act_cost[0] += 2 * w * 0.00109 + 0.1

        def enq(b, c):
            for e in range(len(layout[c][0])):
                sc_ready.append((b, c, e))

        def pump():
            while sc_ready and act_cost[0] < pe_cost[0] + 4.0:
                sc_emit()

        def force(b, c):
            while sc_ready and sc_ready[0][:2] <= (b, c):
                sc_emit()

        def fill(us, fn, *args):
            fn(*args)
            pe_cost[0] += us
            pump()

        # ---- micro-unit worklets ----
        qk_state = {}

        def qk_unit(b, rb, j):
            """Quarter of a q/k projection row-block: chunks 2j, 2j+1."""
            get_batch_tiles(b)
            rsl = slice(rb * 512, (rb + 1) * 512)
            if j == 0:
                qk_state['pqt'] = ppool.tile([128, 512], f32, name="pqt",
                                             tag="pp")
                qk_state['pkt'] = ppool.tile([128, 512], f32, name="pkt",
                                             tag="pp")
            pqt, pkt = qk_state['pqt'], qk_state['pkt']
            for c in (2 * j, 2 * j + 1):
                st = c == 0
                sp = c == NCH - 1
                rhs = xc[:, c, rsl]
                nc.tensor.matmul(pqt, wqT_sb[:, c, :], rhs, start=st, stop=sp)
                nc.tensor.matmul(pkt, wkT_sb[:, c, :], rhs, start=st, stop=sp)
            if j == 3:
                nc.vector.tensor_scalar_add(qt[b][:, rsl], pqt, bq_col)
                nc.vector.tensor_scalar_add(kt_[b][:, rsl], pkt, bk_col)

        def v_unit(b, m0):
            """Two V m-tiles (keys 128*m0 .. 128*m0+256)."""
            for m in (m0, m0 + 1):
                msl = slice(m * 128, (m + 1) * 128)
                pv_ = ppool.tile([128, LD], f32, name="pv", tag="pp")
                for c in range(NCH):
                    nc.tensor.matmul(pv_, xc[:, c, msl], wvT_sb[:, c, :],
                                     start=(c == 0), stop=(c == NCH - 1))
                tv = work.tile([128, LD], f32, name="tv", tag="tv")
                nc.vector.tensor_add(tv, pv_, bv_bc)
                pcol = padf[:, b * NKT + m:b * NKT + m + 1]
                for h in range(2):
                    nc.vector.tensor_scalar_mul(
                        vaug[b][:, h, m, 0:HD], tv[:, h * HD:(h + 1) * HD],
                        pcol)
                    nc.vector.tensor_copy(vaug[b][:, h, m, HD:HD + 1], pcol)

        def pv(b, c):
            entries, _ = layout[c]
            est = ests[b][c]
            for h in range(2):
                po = pvpool.tile([128, 512], f32, name=f"po{h}", tag="po")
                pos[(b, c, h)] = po
                last = 4 * c + 3
                for kt, w, qs, off in entries:
                    po_off = qs - 512 * c
                    nc.tensor.matmul(po[0:HD + 1, po_off:po_off + w],
                                     vaug[b][:, h, kt, :],
                                     est[:, h, off:off + w],
                                     start=(kt == 0), stop=(kt == last))
                den = recp.tile([1, 512], f32, name="den", tag=f"den{h}")
                nc.vector.tensor_copy(den, po[HD:HD + 1, :])
                rec = recp.tile([1, 512], f32, name="rec", tag=f"rec{h}")
                nc.vector.reciprocal_approx_fast(rec, den)
                recb = recp.tile([1, 512], bf16, name="recb",
                                 tag=f"recb{h}", bufs=2)
                nc.vector.tensor_copy(recb, rec)
                recbs[(b, c, h)] = recb

        def div(b, c):
            for h in range(2):
                nc.tensor.matmul(pos[(b, c, h)][64:128, :], ones64,
                                 recbs[(b, c, h)], start=True, stop=True,
                                 skip_group_check=True)
            for h in range(2):
                po = pos[(b, c, h)]
                rbc = recp.tile([HD, 512], bf16, name="rbc", tag=f"rbc{h}")
                nc.vector.tensor_copy(rbc, po[64:128, :])
                nc.vector.tensor_mul(
                    stage[b][h * HD:(h + 1) * HD, 512 * c:512 * (c + 1)],
                    po[0:HD, :], rbc)

        def a2a(b, h2):
            q0, q1 = 1024 * h2, 1024 * (h2 + 1)
            nq = (q1 - q0) // NCORES
            a2a_in = dpool.tile([NCORES * 128, nq], bf16,
                                name=f"a2ai{b}{h2}", tag="a2ai")
            nc.gpsimd.dma_start(
                a2a_in.rearrange("(j p) r -> p j r", p=128),
                stage[b][:, q0:q1].rearrange("p (j r) -> p j r", j=NCORES))
            a2a_out = dpool.tile([NCORES * 128, nq], bf16,
                                 name=f"a2ao{b}{h2}", tag="a2ao")
            nc.gpsimd.collective_compute(
                "AllToAll", ALU.bypass,
                replica_groups=[list(range(NCORES))],
                ins=[a2a_in.opt()], outs=[a2a_out.opt()])
            a2a_outs[(b, h2)] = a2a_out

        op_state = {}

        def op_unit(b, h2, n):
            """Half an output-projection chunk (512 of 1024 out dims)."""
            if n == 0:
                a2a_sb = stg.tile([128, NCORES, 128], bf16,
                                  name=f"a2as{b}{h2}", tag="a2as", bufs=3)
                nc.sync.dma_start(
                    a2a_sb,
                    a2a_outs[(b, h2)].rearrange("(j p) r -> p j r", p=128))
                op_state[(b, h2)] = a2a_sb
            a2a_sb = op_state[(b, h2)]
            r0 = (2 * b + h2) * 128
            pout = ppool.tile([128, 512], f32, name="pout", tag="pp")
            for c in range(NCH):
                nc.tensor.matmul(
                    pout, a2a_sb[:, c, :],
                    woT_sb[:, c, n * 512:(n + 1) * 512],
                    start=(c == 0), stop=(c == NCH - 1))
            ot = work.tile([128, 512], f32, name="ot", tag="ot")
            nc.vector.tensor_add(ot, pout, bo_bc[:, n * 512:(n + 1) * 512])
            nc.sync.dma_start(
                out.ap()[r0:r0 + 128, n * 512:(n + 1) * 512], ot)

        # ---- emission schedule ----
        QKU, VU, OPU, DIVU = 1.05, 1.1, 2.1, 0.55

        # batch-0 projections: V units follow each qk row-block so the
        # xc region's batch-1 reload (issued right behind, Sync queue)
        # unblocks as early as possible
        for rb in range(4):
            for j in range(4):
                fill(QKU, qk_unit, 0, rb, j)
            enq(0, rb)
            fill(VU, v_unit, 0, 4 * rb)
            fill(VU, v_unit, 0, 4 * rb + 2)
            xc_load_rb(1, rb)

        for j in range(4):
            fill(QKU, qk_unit, 1, 0, j)
        enq(1, 0)
        force(0, 0)
        fill(1.3, pv, 0, 0)
        for j in range(4):
            fill(QKU, qk_unit, 1, 1, j)
        enq(1, 1)
        fill(DIVU, div, 0, 0)
        force(0, 1)
        fill(3.4, pv, 0, 1)
        for j in range(4):
            fill(QKU, qk_unit, 1, 2, j)
        enq(1, 2)
        fill(DIVU, div, 0, 1)
        a2a(0, 0)
        force(0, 2)
        fill(5.5, pv, 0, 2)
        for j in range(4):
            fill(QKU, qk_unit, 1, 3, j)
        enq(1, 3)
        fill(DIVU, div, 0, 2)
        for m0 in range(0, 6, 2):
            fill(VU, v_unit, 1, m0)
        force(0, 3)
        fill(7.6, pv, 0, 3)
        fill(VU, v_unit, 1, 6)
        fill(DIVU, div, 0, 3)
        a2a(0, 1)
        for m0 in range(8, NKT, 2):
            fill(VU, v_unit, 1, m0)
        force(1, 0)
        fill(1.3, pv, 1, 0)
        fill(DIVU, div, 1, 0)
        force(1, 1)
        fill(3.4, pv, 1, 1)
        fill(DIVU, div, 1, 1)
        a2a(1, 0)
        force(1, 2)
        fill(5.5, pv, 1, 2)
        fill(DIVU, div, 1, 2)
        fill(OPU, op_unit, 0, 0, 0)
        fill(OPU, op_unit, 0, 0, 1)
        fill(OPU, op_unit, 0, 1, 0)
        fill(OPU, op_unit, 0, 1, 1)
        force(1, 3)
        fill(7.6, pv, 1, 3)
        fill(DIVU, div, 1, 3)
        a2a(1, 1)
        fill(OPU, op_unit, 1, 0, 0)
        fill(OPU, op_unit, 1, 0, 1)
        fill(OPU, op_unit, 1, 1, 0)
        fill(OPU, op_unit, 1, 1, 1)

        assert not sc_ready

    nc.finalize()
    return nc


def _get_nc():
    with _lock:
        if "nc" not in _cache:
            _cache["nc"] = _build_nc()
        return _cache["nc"]


def _shard_inputs(x, input_ids, Wq, bq, Wk, bk, Wv, bv, Wo, bo):
    import ml_dtypes
    bf16 = ml_dtypes.bfloat16

    x = np.asarray(x, dtype=np.float32)
    xT = np.ascontiguousarray(x.reshape(R, D).T).astype(bf16)
    woT = np.ascontiguousarray(np.asarray(Wo, dtype=np.float32).T).astype(bf16)
    bo_f = np.asarray(bo, dtype=np.float32)
    ids = np.asarray(input_ids).astype(np.int32)
    # ids_r[p, b*NKT + t] = input_ids[b, t*128 + p]
    ids_r = np.ascontiguousarray(ids.reshape(B, NKT, 128).transpose(2, 0, 1)
                                 .reshape(128, B * NKT))
    Wq = np.asarray(Wq, dtype=np.float32)
    Wk = np.asarray(Wk, dtype=np.float32)
    Wv = np.asarray(Wv, dtype=np.float32)
    bq = np.asarray(bq, dtype=np.float32)
    bk = np.asarray(bk, dtype=np.float32)
    bv = np.asarray(bv, dtype=np.float32)

    in_maps = []
    for c in range(NCORES):
        sl = slice(c * LD, (c + 1) * LD)
        in_maps.append({
            "xT": xT,
            "wqT": np.ascontiguousarray(Wq[sl].T).astype(bf16),
            "wkT": np.ascontiguousarray(Wk[sl].T).astype(bf16),
            "wvT": np.ascontiguousarray(Wv[sl].T).astype(bf16),
            "woT": woT,
            "bq": bq[sl].copy(),
            "bk": bk[sl].copy(),
            "bv": bv[sl].copy(),
            "bo": bo_f,
            "ids": ids_r,
        })
    return in_maps


def run(trace=False, **inputs):
    """Run the kernel; returns (output, BassKernelResults)."""
    from concourse.bass_utils import run_bass_kernel_spmd

    nc = _get_nc()
    in_maps = _shard_inputs(**inputs)
    res = run_bass_kernel_spmd(nc, in_maps, core_ids=list(range(NCORES)),
                               trace=trace)
    full = np.empty((B, S, D), dtype=np.float32)
    for j in range(NCORES):
        o = np.asarray(res.results[j]["out"], dtype=np.float32)
        for b in range(B):
            for h2 in range(2):
                full[b, 1024 * h2 + 128 * j:1024 * h2 + 128 * (j + 1), :] = \
                    o[(2 * b + h2) * 128:(2 * b + h2 + 1) * 128, :]
    return full, res


def kernel(**inputs) -> np.ndarray:
    full, _ = run(trace=False, **inputs)
    return full
